# revision 1
# baseline (speedup 1.0000x reference)
"""GQA FlashAttention (RMSNorm QK + RoPE, causal) on 8 TRN2 NeuronCores.

Sharding: tensor-parallel over heads for QKV projection + attention
(core c owns q-heads 4c..4c+3 and kv-head c — the GQA group is fully
local, so attention needs no collective). A single AllToAll re-shards
the attention output from head-parallel to seq-row-parallel, after
which each core computes its 256 output rows against the full Wo
(no all-reduce). Softmax uses the unnormalized-exp trick: denominators
come free from a ones-column appended to V, and the division is applied
to the small attention output after the PV matmul.

All matmuls run in float32r (fp32 storage, ~4x fp32 PE rate; measured
same precision as the fp32 PE path). Everything is computed in the
transposed layout (head_dim on partitions) so the scores output IS the
P^T operand the PV matmul needs — zero transposes in the attention
inner loop.
"""

import sys

sys.path.insert(0, "/opt/trn_rl_repo")

import numpy as np
import concourse.bass as bass  # noqa: F401  (engine types referenced via nc)
import concourse.tile as tile
from concourse import mybir, bacc
from concourse.bass_utils import run_bass_kernel_spmd
from concourse.masks import make_identity

N_CORES = 8
D_IN = 2048
SEQ = 2048
N_HEADS = 32
N_KV = 8
HD = 64
HPC = N_HEADS // N_CORES  # 4 q heads per core
EPS = 1e-6
NEG = -1.0e9

F32 = mybir.dt.float32
F32R = mybir.dt.float32r

KT_TILES = D_IN // 128  # 16 contraction tiles for projections
QB = 512  # q block (matmul moving dim)
NQB = SEQ // QB  # 4
NKT = SEQ // 128  # 16 kv tiles
ROWS_PER_CORE = SEQ // N_CORES  # 256


def _build():
    nc = bacc.Bacc(num_devices=N_CORES)

    xT = nc.dram_tensor("xT", [D_IN, SEQ], F32R, kind="ExternalInput")
    wq = nc.dram_tensor("wq", [D_IN, HPC * HD], F32R, kind="ExternalInput")
    wkv = nc.dram_tensor("wkv", [D_IN, 2 * HD], F32R, kind="ExternalInput")
    wo = nc.dram_tensor("wo", [D_IN, D_IN], F32R, kind="ExternalInput")
    cosT2 = nc.dram_tensor("cosT2", [128, SEQ], F32, kind="ExternalInput")
    sinT2 = nc.dram_tensor("sinT2", [128, SEQ], F32, kind="ExternalInput")
    qw2 = nc.dram_tensor("qw2", [128, 1], F32, kind="ExternalInput")
    kw = nc.dram_tensor("kw", [64, 1], F32, kind="ExternalInput")
    tri = nc.dram_tensor("tri", [128, 128], F32, kind="ExternalInput")
    sel = nc.dram_tensor("sel", [4 * N_CORES, 2 * N_CORES, 128], F32R, kind="ExternalInput")
    onesblk_in = nc.dram_tensor("onesblk", [128, 128], F32R, kind="ExternalInput")
    onescol_in = nc.dram_tensor("onescol", [128, 1], F32R, kind="ExternalInput")

    out = nc.dram_tensor("out", [ROWS_PER_CORE, D_IN], F32, kind="ExternalOutput")

    with tile.TileContext(nc) as tc:
        with (
            tc.tile_pool(name="persist", bufs=1) as pers,
            tc.tile_pool(name="dram", bufs=1, space="DRAM") as dram,
        ):
            # ---- persistent SBUF ----
            wq_sb = pers.tile([128, KT_TILES, HPC * HD], F32R)  # 2 MB
            nc.sync.dma_start(
                wq_sb[:], wq.rearrange("(ko p) m -> p ko m", p=128)
            )
            wkv_sb = pers.tile([128, KT_TILES, 2 * HD], F32R)  # 1 MB
            nc.sync.dma_start(
                wkv_sb[:], wkv.rearrange("(ko p) m -> p ko m", p=128)
            )
            cos_sb = pers.tile([128, SEQ], F32)
            sin_sb = pers.tile([128, SEQ], F32)
            nc.sync.dma_start(cos_sb[:], cosT2[:])
            nc.sync.dma_start(sin_sb[:], sinT2[:])
            qw_sb = pers.tile([128, 1], F32)
            kw_sb = pers.tile([64, 1], F32)
            nc.sync.dma_start(qw_sb[:], qw2[:])
            nc.sync.dma_start(kw_sb[:], kw[:])
            tri_sb = pers.tile([128, 128], F32)
            nc.sync.dma_start(tri_sb[:], tri[:])
            eps_sb = pers.tile([128, 1], F32)
            nc.vector.memset(eps_sb[:], EPS)
            sel_sb = pers.tile([4 * N_CORES, 2 * N_CORES, 128], F32R)
            nc.sync.dma_start(sel_sb[:], sel[:])

            ident = pers.tile([128, 128], F32)
            make_identity(nc, ident[:])

            # block-diagonal ones (two 64x64 blocks) for per-head sumsq+broadcast
            onesblk = pers.tile([128, 128], F32R)
            nc.sync.dma_start(onesblk[:], onesblk_in[:])

            # QT per head at base partition 0: [64, 4 heads, SEQ]
            qt_sb = pers.tile([64, HPC, SEQ], F32R)  # 2 MB
            kt_sb = pers.tile([64, SEQ], F32R)  # 0.5 MB
            vaug_sb = pers.tile([128, NKT, HD + 1], F32R)  # 0.53 MB
            for _t in range(NKT):
                nc.sync.dma_start(vaug_sb[:, _t, HD : HD + 1], onescol_in[:])

            # DRAM scratch for the AllToAll
            a2a_in = dram.tile([N_CORES, HPC * HD + HPC, ROWS_PER_CORE], F32)
            a2a_out = dram.tile([N_CORES, HPC * HD + HPC, ROWS_PER_CORE], F32)

            # ================= Phase 1: projections + norm + rope =============
            with (
                tc.tile_pool(name="xt", bufs=4) as xp,
                tc.tile_pool(name="p1ps", bufs=2, space="PSUM") as psA,
                tc.tile_pool(name="p1sb", bufs=3) as t1,
            ):
                for j in range(NQB):
                    sl = slice(QB * j, QB * j + QB)
                    acc = [
                        psA.tile([128, QB], F32, tag="acc0", name=f"acc0_{j}"),
                        psA.tile([128, QB], F32, tag="acc1", name=f"acc1_{j}"),
                        psA.tile([128, QB], F32, tag="acc2", name=f"acc2_{j}"),
                    ]
                    for k in range(KT_TILES):
                        xt = xp.tile([128, QB], F32R, tag="xt")
                        nc.sync.dma_start(
                            xt[:], xT[128 * k : 128 * k + 128, sl]
                        )
                        st = k == 0
                        sp = k == KT_TILES - 1
                        nc.tensor.matmul(
                            acc[0][:], wq_sb[:, k, 0:128], xt[:], start=st, stop=sp
                        )
                        nc.tensor.matmul(
                            acc[1][:], wq_sb[:, k, 128:256], xt[:], start=st, stop=sp
                        )
                        nc.tensor.matmul(
                            acc[2][:], wkv_sb[:, k, :], xt[:], start=st, stop=sp
                        )

                    for idx in range(3):
                        raw = acc[idx]
                        is_kv = idx == 2
                        rows = slice(0, 64) if is_kv else slice(0, 128)
                        # sumsq broadcast per head (block-diag ones matmul)
                        sq = t1.tile([128, QB], F32R, tag="sq")
                        nc.scalar.square(sq[:], raw[:])
                        psn = psA.tile([128, QB], F32, tag="norm", bufs=1)
                        nc.tensor.matmul(
                            psn[:], onesblk[:], sq[:], start=True, stop=True
                        )
                        rcp = t1.tile([128, QB], F32, tag="rcp")
                        nc.scalar.activation(
                            out=rcp[rows, :],
                            in_=psn[rows, :],
                            func=mybir.ActivationFunctionType.Sqrt,
                            bias=eps_sb[rows, :],
                            scale=1.0 / HD,
                        )
                        nc.vector.reciprocal(rcp[rows, :], rcp[rows, :])
                        # normalized = raw * rcp * norm_w
                        tn = t1.tile([128, QB], F32, tag="tn")
                        nc.vector.tensor_mul(tn[rows, :], raw[rows, :], rcp[rows, :])
                        if is_kv:
                            nc.vector.tensor_scalar_mul(
                                tn[0:64, :], tn[0:64, :], kw_sb[:]
                            )
                        else:
                            nc.vector.tensor_scalar_mul(tn[:], tn[:], qw_sb[:])
                        # rope: rot = [-t[32:64], t[0:32]] per 64-row head
                        rot = t1.tile([128, QB], F32, tag="rot")
                        nheads_here = 1 if is_kv else 2
                        for b in range(nheads_here):
                            o = 64 * b
                            nc.vector.tensor_scalar_mul(
                                rot[o : o + 32, :], tn[o + 32 : o + 64, :], -1.0
                            )
                            nc.vector.tensor_copy(
                                rot[o + 32 : o + 64, :], tn[o : o + 32, :]
                            )
                        if is_kv:
                            dst = kt_sb[:, sl]
                            nc.vector.tensor_mul(dst, tn[0:64, :], cos_sb[0:64, sl])
                            nc.vector.tensor_mul(
                                rot[0:64, :], rot[0:64, :], sin_sb[0:64, sl]
                            )
                            nc.vector.tensor_add(dst, dst, rot[0:64, :])
                            # V rows: evict + transpose to natural layout
                            vt = t1.tile([64, QB], F32, tag="vt")
                            nc.scalar.copy(vt[:], raw[64:128, :])
                            for ttl in range(QB // 128):
                                tg = (QB // 128) * j + ttl
                                psv = psA.tile([128, 64], F32, tag="vtr", bufs=1)
                                nc.tensor.transpose(
                                    psv[:],
                                    vt[:, 128 * ttl : 128 * ttl + 128],
                                    ident[0:64, 0:64],
                                )
                                nc.scalar.copy(vaug_sb[:, tg, 0:HD], psv[:])
                        else:
                            tmpc = t1.tile([128, QB], F32, tag="tmpc")
                            nc.vector.tensor_mul(tmpc[:], tn[:], cos_sb[:, sl])
                            nc.vector.tensor_mul(rot[:], rot[:], sin_sb[:, sl])
                            for b in range(2):
                                nc.vector.tensor_add(
                                    qt_sb[:, 2 * idx + b, sl],
                                    tmpc[64 * b : 64 * b + 64, :],
                                    rot[64 * b : 64 * b + 64, :],
                                )

            # ================= Phase 3: attention =============================
            with (
                tc.tile_pool(name="p3ps", bufs=3, space="PSUM") as psB,
                tc.tile_pool(name="p3pv", bufs=2, space="PSUM") as psPV,
                tc.tile_pool(name="p3sb", bufs=3) as t3,
            ):
                for h in range(HPC):
                    for j in range(NQB):
                        ntile = (QB // 128) * (j + 1)
                        pv = psPV.tile([128, QB], F32, tag="pv")
                        for t in range(ntile):
                            diag_m = t - (QB // 128) * j
                            ks = slice(128 * t, 128 * t + 128)
                            if diag_m < 0:
                                qs = slice(QB * j, QB * j + QB)
                                n0 = 0
                            else:
                                n0 = 128 * diag_m
                                qs = slice(QB * j + n0, QB * j + QB)
                            ps_s = psB.tile([128, QB], F32, tag="sc")
                            nc.tensor.matmul(
                                ps_s[:, 0 : QB - n0],
                                kt_sb[:, ks],
                                qt_sb[:, h, qs],
                                start=True,
                                stop=True,
                            )
                            if diag_m >= 0:
                                nc.vector.tensor_add(
                                    ps_s[:, 0:128], ps_s[:, 0:128], tri_sb[:]
                                )
                            pt = t3.tile([128, QB], F32R, tag="pt")
                            nc.scalar.activation(
                                out=pt[:, 0 : QB - n0],
                                in_=ps_s[:, 0 : QB - n0],
                                func=mybir.ActivationFunctionType.Exp,
                                scale=0.125,
                            )
                            nc.tensor.matmul(
                                pv[0:65, n0:QB],
                                vaug_sb[:, t, :],
                                pt[:, 0 : QB - n0],
                                start=(t == 0),
                                stop=(t == ntile - 1),
                            )
                        att = t3.tile([65, QB], F32, tag="att")
                        nc.scalar.copy(att[:], pv[0:65, :])
                        for s in range(QB // ROWS_PER_CORE):
                            shard = (QB // ROWS_PER_CORE) * j + s
                            cs = slice(ROWS_PER_CORE * s, ROWS_PER_CORE * (s + 1))
                            nc.sync.dma_start(
                                a2a_in[shard, 64 * h : 64 * h + 64, :],
                                att[0:64, cs],
                            )
                            nc.sync.dma_start(
                                a2a_in[shard, HPC * 64 + h, :], att[64:65, cs]
                            )

            # ================= Phase 4: AllToAll ==============================
            nc.gpsimd.collective_compute(
                "AllToAll",
                mybir.AluOpType.bypass,
                replica_groups=[list(range(N_CORES))],
                ins=[a2a_in[:].opt()],
                outs=[a2a_out[:].opt()],
            )

            # ================= Phase 5: out projection ========================
            with (
                tc.tile_pool(name="p5ps", bufs=2, space="PSUM") as psC,
                tc.tile_pool(name="p5bc", bufs=2, space="PSUM") as psD,
                tc.tile_pool(name="wo", bufs=8) as wop,
                tc.tile_pool(name="p5sb", bufs=4) as t5,
                tc.tile_pool(name="an", bufs=1) as anp,
            ):
                R = ROWS_PER_CORE
                dsb_raw = t5.tile([4 * N_CORES, R], F32, tag="denraw")
                for g in range(N_CORES):
                    nc.sync.dma_start(
                        dsb_raw[4 * g : 4 * g + 4, :],
                        a2a_out[g, HPC * 64 : HPC * 64 + 4, :],
                    )
                nc.vector.reciprocal(dsb_raw[:], dsb_raw[:])
                dsb = t5.tile([4 * N_CORES, R], F32R, tag="den")
                nc.vector.tensor_copy(dsb[:], dsb_raw[:])

                an_sb = anp.tile([128, 2 * N_CORES, R], F32R)  # normalized attnT
                for g in range(N_CORES):
                    for half in range(2):
                        a_raw = t5.tile([128, R], F32, tag="araw")
                        nc.sync.dma_start(
                            a_raw[:], a2a_out[g, 128 * half : 128 * half + 128, :]
                        )
                        bc = psD.tile([128, R], F32, tag="bc")
                        nc.tensor.matmul(
                            bc[:],
                            sel_sb[:, 2 * g + half, :],
                            dsb[:],
                            start=True,
                            stop=True,
                        )
                        nc.vector.tensor_mul(
                            an_sb[:, 2 * g + half, :], a_raw[:], bc[:]
                        )

                NB_OUT = D_IN // 512  # 4
                for nb in range(NB_OUT):
                    osl = slice(512 * nb, 512 * nb + 512)
                    po = [
                        psC.tile([128, 512], F32, tag="o0", name=f"o0_{nb}"),
                        psC.tile([128, 512], F32, tag="o1", name=f"o1_{nb}"),
                    ]
                    for gh in range(2 * N_CORES):
                        wt = wop.tile([128, 512], F32R, tag="wo")
                        nc.sync.dma_start(
                            wt[:], wo[128 * gh : 128 * gh + 128, osl]
                        )
                        for qt in range(2):
                            nc.tensor.matmul(
                                po[qt][:],
                                an_sb[:, gh, 128 * qt : 128 * qt + 128],
                                wt[:],
                                start=(gh == 0),
                                stop=(gh == 2 * N_CORES - 1),
                            )
                    for qt in range(2):
                        osb = t5.tile([128, 512], F32, tag="osb")
                        nc.scalar.copy(osb[:], po[qt][:])
                        nc.sync.dma_start(
                            out[128 * qt : 128 * qt + 128, osl], osb[:]
                        )

    nc.compile()
    return nc


_NC_CACHE = None


def _get_nc():
    global _NC_CACHE
    if _NC_CACHE is None:
        _NC_CACHE = _build()
    return _NC_CACHE


def _make_in_maps(x, cos, sin, wq, wk, wv, wo, q_norm_w, k_norm_w):
    x = np.asarray(x, dtype=np.float32)
    cos = np.asarray(cos, dtype=np.float32)
    sin = np.asarray(sin, dtype=np.float32)
    wq = np.asarray(wq, dtype=np.float32)
    wk = np.asarray(wk, dtype=np.float32)
    wv = np.asarray(wv, dtype=np.float32)
    wo = np.asarray(wo, dtype=np.float32)
    q_norm_w = np.asarray(q_norm_w, dtype=np.float32)
    k_norm_w = np.asarray(k_norm_w, dtype=np.float32)

    xT = np.ascontiguousarray(x[0].T)  # [D_IN, SEQ]
    cosT2 = np.ascontiguousarray(np.vstack([cos.T, cos.T]))  # [128, SEQ]
    sinT2 = np.ascontiguousarray(np.vstack([sin.T, sin.T]))
    qw2 = np.ascontiguousarray(np.concatenate([q_norm_w, q_norm_w])[:, None])
    kw1 = np.ascontiguousarray(k_norm_w[:, None])
    ii, jj = np.meshgrid(np.arange(128), np.arange(128), indexing="ij")
    tri = np.where(ii <= jj, 0.0, NEG).astype(np.float32)  # keep kv<=q
    onesblk = np.zeros((128, 128), np.float32)
    onesblk[0:64, 0:64] = 1.0
    onesblk[64:128, 64:128] = 1.0
    onescol = np.ones((128, 1), np.float32)
    sel = np.zeros((4 * N_CORES, 2 * N_CORES, 128), np.float32)
    for g in range(N_CORES):
        for half in range(2):
            for m in range(128):
                sel[4 * g + 2 * half + m // 64, 2 * g + half, m] = 1.0

    in_maps = []
    for c in range(N_CORES):
        wq_c = np.ascontiguousarray(wq[:, 256 * c : 256 * c + 256])
        wkv_c = np.ascontiguousarray(
            np.concatenate(
                [wk[:, 64 * c : 64 * c + 64], wv[:, 64 * c : 64 * c + 64]], axis=1
            )
        )
        in_maps.append(
            {
                "xT": xT,
                "wq": wq_c,
                "wkv": wkv_c,
                "wo": wo,
                "cosT2": cosT2,
                "sinT2": sinT2,
                "qw2": qw2,
                "kw": kw1,
                "tri": tri,
                "sel": sel,
                "onesblk": onesblk,
                "onescol": onescol,
            }
        )
    return in_maps


def kernel(x, cos, sin, wq, wk, wv, wo, q_norm_w, k_norm_w):
    in_maps = _make_in_maps(x, cos, sin, wq, wk, wv, wo, q_norm_w, k_norm_w)
    nc = _get_nc()
    res = run_bass_kernel_spmd(nc, in_maps, core_ids=list(range(N_CORES)))
    rows = [res.results[c]["out"] for c in range(N_CORES)]
    full = np.concatenate(rows, axis=0)  # [SEQ, D_IN]
    return full.reshape(1, SEQ, D_IN).astype(np.float32)



# revision 13
# speedup vs baseline: 1.3068x; 1.3068x over previous
"""GQA FlashAttention (RMSNorm QK + RoPE, causal) on 8 TRN2 NeuronCores.

Sharding: tensor-parallel over heads (core c owns q-heads 4c..4c+3 and
kv-head c; the GQA group is fully local so attention needs no
collective). The attention output is re-sharded head-parallel ->
row-parallel with TWO AllToAlls (one per 1024-row half) so the first
collective and the first half of the out-projection overlap with the
attention compute of the second half. Each core then multiplies its
256 output rows (2 x 128) against the full Wo held in SBUF (bf16,
prefetched during the projection phase).

Softmax uses the unnormalized-exp trick: denominators come free from a
ones-column appended to V, and the division is applied after the
AllToAll via a select-matmul broadcast. rsqrt/reciprocal are computed
as exp(-a*ln(x)) so the scalar engine stays on ONE activation table
set (natural_log_exp_and_others) for the whole kernel, and the slow
DVE iterative-divide reciprocal is never used.

All matmuls run in bf16 (fp32 PSUM accumulate). Everything is computed
in the transposed layout (head_dim on partitions) so the scores output
IS the P^T operand the PV matmul needs - zero transposes in the
attention inner loop. The RMSNorm weights and the rotate-half signs
are folded into per-row cos/sin tables host-side.
"""

import sys

sys.path.insert(0, "/opt/trn_rl_repo")

import ml_dtypes
import numpy as np
import concourse.bass as bass  # noqa: F401
import concourse.tile as tile
from concourse import mybir, bacc
from concourse.bass_utils import run_bass_kernel_spmd

N_CORES = 8
D_IN = 2048
SEQ = 2048
N_HEADS = 32
N_KV = 8
HD = 64
HPC = N_HEADS // N_CORES  # 4 q heads per core
EPS = 1e-6
NEG = -1.0e9

F32 = mybir.dt.float32
BF16 = mybir.dt.bfloat16
BFNP = ml_dtypes.bfloat16

KT = D_IN // 128  # 16 contraction tiles for projections
QB = 512  # q block
NQB = SEQ // QB  # 4
NKT = SEQ // 128  # 16 kv tiles
ROWS = 128  # output rows per core per half
AF = mybir.ActivationFunctionType


def _build():
    nc = bacc.Bacc(num_devices=N_CORES)

    # x re-tiled host-side: xq[p, j, k, c] = x[512j+c, 128k+p]
    xq = nc.dram_tensor("xq", [128, NQB, KT, QB], BF16, kind="ExternalInput")
    wq = nc.dram_tensor("wq", [128, KT, HPC * HD], BF16, kind="ExternalInput")
    wkv = nc.dram_tensor("wkv", [128, KT, 2 * HD], BF16, kind="ExternalInput")
    wo = nc.dram_tensor("wo", [128, KT, D_IN], BF16, kind="ExternalInput")
    cosq = nc.dram_tensor("cosq", [128, SEQ], BF16, kind="ExternalInput")
    sinq = nc.dram_tensor("sinq", [128, SEQ], BF16, kind="ExternalInput")
    cosk = nc.dram_tensor("cosk", [64, SEQ], BF16, kind="ExternalInput")
    sink = nc.dram_tensor("sink", [64, SEQ], BF16, kind="ExternalInput")
    tri = nc.dram_tensor("tri", [128, 128], F32, kind="ExternalInput")
    onesblk_in = nc.dram_tensor("onesblk", [128, 128], BF16, kind="ExternalInput")
    onescol_in = nc.dram_tensor("onescol", [128, 1], BF16, kind="ExternalInput")
    ident_in = nc.dram_tensor("ident", [64, 64], BF16, kind="ExternalInput")
    sel = nc.dram_tensor("sel", [4 * N_CORES, 2 * N_CORES, 128], BF16, kind="ExternalInput")

    out = nc.dram_tensor("out", [2 * ROWS, D_IN], F32, kind="ExternalOutput")

    with tile.TileContext(nc) as tc:
        with (
            tc.tile_pool(name="pers", bufs=1) as pers,
            tc.tile_pool(name="dram", bufs=1, space="DRAM") as dram,
            tc.tile_pool(name="xp", bufs=2) as xp,
            tc.tile_pool(name="psproj", bufs=1, space="PSUM") as psProj,
            tc.tile_pool(name="psaux", bufs=1, space="PSUM") as psAux,
            tc.tile_pool(name="pspo", bufs=1, space="PSUM") as psPo,
            tc.tile_pool(name="pssc", bufs=2, space="PSUM") as psSc,
            tc.tile_pool(name="pspv", bufs=1, space="PSUM") as psPv,
            tc.tile_pool(name="t1", bufs=2) as t1,
            tc.tile_pool(name="t3", bufs=3) as t3,
            tc.tile_pool(name="t5", bufs=2) as t5,
        ):
            # ---------------- persistent SBUF ----------------
            wq_sb = pers.tile([128, KT, HPC * HD], BF16)  # 1 MB
            nc.sync.dma_start(wq_sb[:], wq[:])
            wkv_sb = pers.tile([128, KT, 2 * HD], BF16)  # 0.5 MB
            nc.sync.dma_start(wkv_sb[:], wkv[:])
            cosq_sb = pers.tile([128, SEQ], BF16)
            sinq_sb = pers.tile([128, SEQ], BF16)
            cosk_sb = pers.tile([64, SEQ], BF16)
            sink_sb = pers.tile([64, SEQ], BF16)
            nc.sync.dma_start(cosq_sb[:], cosq[:])
            nc.sync.dma_start(sinq_sb[:], sinq[:])
            nc.sync.dma_start(cosk_sb[:], cosk[:])
            nc.sync.dma_start(sink_sb[:], sink[:])
            tri_sb = pers.tile([128, 128], F32)
            nc.sync.dma_start(tri_sb[:], tri[:])
            onesblk = pers.tile([128, 128], BF16)
            nc.sync.dma_start(onesblk[:], onesblk_in[:])
            ident = pers.tile([64, 64], BF16)
            nc.sync.dma_start(ident[:], ident_in[:])
            sel_sb = pers.tile([4 * N_CORES, 2 * N_CORES, 128], BF16)
            nc.sync.dma_start(sel_sb[:], sel[:])

            eps_sb = pers.tile([128, 1], F32)
            nc.vector.memset(eps_sb[:], EPS)

            qt_sb = pers.tile([64, HPC, SEQ], BF16)  # 1 MB
            kt_sb = pers.tile([64, SEQ], BF16)
            vaug_sb = pers.tile([128, NKT, HD + 1], BF16)
            for _t in range(NKT):
                nc.sync.dma_start(vaug_sb[:, _t, HD : HD + 1], onescol_in[:])

            # first x block before the big wo prefetch
            xts = [None] * NQB
            xts[0] = xp.tile([128, KT, QB], BF16, tag="xt", name="xt_0")
            nc.sync.dma_start(xts[0][:], xq[:, 0])

            # wo prefetch (8 MB bf16), in 4 chunks
            wo_sb = pers.tile([128, KT, D_IN], BF16)
            for wch in range(4):
                nc.sync.dma_start(
                    wo_sb[:, 4 * wch : 4 * wch + 4, :], wo[:, 4 * wch : 4 * wch + 4, :]
                )

            # DRAM scratch for the two AllToAlls (head-major rows: [h][65])
            a2a_in = [
                dram.tile([N_CORES, HPC, HD + 1, ROWS], BF16, name=f"a2a_in{i}")
                for i in range(2)
            ]
            a2a_out = [
                dram.tile([N_CORES, HPC, HD + 1, ROWS], BF16, name=f"a2a_out{i}")
                for i in range(2)
            ]

            an_sb = [None, None]  # normalized attnT per half

            def p5_prep(half):
                """Denominator reciprocals + normalized attnT for one half."""
                dsb = t5.tile([4 * N_CORES, ROWS], BF16, tag="dsb", name=f"dsb_{half}")
                for g in range(N_CORES):
                    nc.sync.dma_start(
                        dsb[4 * g : 4 * g + 4, :], a2a_out[half][g, :, HD, :]
                    )
                rcd = t5.tile([4 * N_CORES, ROWS], F32, tag="rcd", name=f"rcd_{half}")
                nc.scalar.activation(rcd[:], dsb[:], AF.Ln)
                drc = t5.tile([4 * N_CORES, ROWS], BF16, tag="drc", name=f"drc_{half}")
                nc.scalar.activation(drc[:], rcd[:], AF.Exp, scale=-1.0)

                an = pers.tile([128, 2 * N_CORES, ROWS], BF16, name=f"an_sb_{half}")
                an_sb[half] = an
                for g in range(N_CORES):
                    for u in range(2):
                        gh = 2 * g + u
                        araw = t5.tile([128, ROWS], BF16, tag="araw", name=f"araw_{half}_{gh}", bufs=4)
                        for hh in (2 * u, 2 * u + 1):
                            nc.sync.dma_start(
                                araw[64 * (hh % 2) : 64 * (hh % 2) + 64, :],
                                a2a_out[half][g, hh, 0:HD, :],
                            )
                        bc = psAux.tile([128, ROWS], F32, tag="aux", name=f"bc_{half}_{gh}")
                        nc.tensor.matmul(
                            bc[:], sel_sb[:, gh, :], drc[:], start=True, stop=True
                        )
                        nc.vector.tensor_mul(an[:, gh, :], araw[:], bc[:])

            def p5_matmul(half, nb):
                """One 512-col block of the out-projection for one half."""
                osl = slice(512 * nb, 512 * nb + 512)
                po = psPo.tile([128, 512], F32, tag="po", name=f"po_{half}_{nb}")
                an = an_sb[half]
                for gh in range(2 * N_CORES):
                    nc.tensor.matmul(
                        po[:],
                        an[:, gh, :],
                        wo_sb[:, gh, osl],
                        start=(gh == 0),
                        stop=(gh == 2 * N_CORES - 1),
                    )
                osb = t5.tile([128, 512], F32, tag="osb", name=f"osb_{half}_{nb}")
                nc.vector.tensor_copy(osb[:], po[:])
                nc.sync.dma_start(out[128 * half : 128 * half + 128, osl], osb[:])

            for j in range(NQB):
                sl = slice(QB * j, QB * j + QB)
                if j > 0:
                    xts[j] = xp.tile([128, KT, QB], BF16, tag="xt", name=f"xt_{j}")
                    nc.sync.dma_start(xts[j][:], xq[:, j])
                xt = xts[j]

                # ---------- projections ----------
                acc = [
                    psProj.tile([128, QB], F32, tag="acc0", name=f"acc0_{j}"),
                    psProj.tile([128, QB], F32, tag="acc1", name=f"acc1_{j}"),
                    psProj.tile([128, QB], F32, tag="acc2", name=f"acc2_{j}"),
                ]
                for k in range(KT):
                    st = k == 0
                    sp = k == KT - 1
                    nc.tensor.matmul(
                        acc[0][:], wq_sb[:, k, 0:128], xt[:, k, :], start=st, stop=sp
                    )
                    nc.tensor.matmul(
                        acc[1][:], wq_sb[:, k, 128:256], xt[:, k, :], start=st, stop=sp
                    )
                    nc.tensor.matmul(
                        acc[2][:], wkv_sb[:, k, :], xt[:, k, :], start=st, stop=sp
                    )

                # ---------- norm + rope ----------
                for idx in range(3):
                    raw = acc[idx]
                    is_kv = idx == 2
                    nr = 64 if is_kv else 128
                    rows = slice(0, nr)
                    sq = t1.tile([128, QB], BF16, tag="sq", name=f"sq_{j}_{idx}")
                    nc.scalar.activation(sq[rows, :], raw[rows, :], AF.Square)
                    psn = psAux.tile([128, QB], F32, tag="aux", name=f"psn_{j}_{idx}")
                    nc.tensor.matmul(
                        psn[rows, :], onesblk[rows, rows], sq[rows, :],
                        start=True, stop=True,
                    )
                    # rsqrt(ms + eps) = exp(-0.5 * ln(ms + eps)); one ACT table set
                    lnt = t1.tile([128, QB], F32, tag="lnt", name=f"lnt_{j}_{idx}")
                    nc.scalar.activation(
                        lnt[rows, :], psn[rows, :], AF.Ln, bias=eps_sb[rows, :],
                        scale=1.0 / HD,
                    )
                    rcp = t1.tile([128, QB], F32, tag="rcp", name=f"rcp_{j}_{idx}")
                    nc.scalar.activation(rcp[rows, :], lnt[rows, :], AF.Exp, scale=-0.5)
                    tn = t1.tile([128, QB], BF16, tag="tn", name=f"tn_{j}_{idx}")
                    nc.vector.tensor_mul(tn[rows, :], raw[rows, :], rcp[rows, :])
                    # rotate-half (signs folded into the sin tables)
                    rot = t1.tile([128, QB], BF16, tag="rot", name=f"rot_{j}_{idx}")
                    for b in range(1 if is_kv else 2):
                        o = 64 * b
                        nc.vector.tensor_copy(rot[o : o + 32, :], tn[o + 32 : o + 64, :])
                        nc.vector.tensor_copy(rot[o + 32 : o + 64, :], tn[o : o + 32, :])
                    cw = cosk_sb[:, sl] if is_kv else cosq_sb[rows, sl]
                    sw = sink_sb[:, sl] if is_kv else sinq_sb[rows, sl]
                    tmpc = t1.tile([128, QB], BF16, tag="tmpc", name=f"tmpc_{j}_{idx}")
                    nc.vector.tensor_mul(tmpc[rows, :], tn[rows, :], cw)
                    nc.vector.tensor_mul(rot[rows, :], rot[rows, :], sw)
                    if is_kv:
                        nc.vector.tensor_add(kt_sb[:, sl], tmpc[0:64, :], rot[0:64, :])
                        # V: evict + transpose to kv-major layout
                        vt = t1.tile([64, QB], BF16, tag="vt", name=f"vt_{j}")
                        nc.vector.tensor_copy(vt[:], raw[64:128, :])
                        for ttl in range(QB // 128):
                            tg = (QB // 128) * j + ttl
                            psv = psAux.tile(
                                [128, HD], BF16, tag="aux", name=f"psv_{tg}"
                            )
                            nc.tensor.transpose(
                                psv[:], vt[:, 128 * ttl : 128 * ttl + 128], ident[:]
                            )
                            nc.vector.tensor_copy(vaug_sb[:, tg, 0:HD], psv[:])
                    else:
                        for b in range(2):
                            nc.vector.tensor_add(
                                qt_sb[:, 2 * idx + b, sl],
                                tmpc[64 * b : 64 * b + 64, :],
                                rot[64 * b : 64 * b + 64, :],
                            )

                # half-0 normalization prep goes before att2 (a2a0 is done by then)
                if j == 2:
                    p5_prep(0)

                # ---------- attention block j ----------
                ntile = (QB // 128) * (j + 1)
                half = j // 2
                for h in range(HPC):
                    pv = psPv.tile([HD + 1, QB], F32, tag="pv", name=f"pv_{j}_{h}")
                    for t in range(ntile):
                        diag_m = t - (QB // 128) * j
                        ks = slice(128 * t, 128 * t + 128)
                        if diag_m < 0:
                            n0 = 0
                            qs = slice(QB * j, QB * j + QB)
                        else:
                            n0 = 128 * diag_m
                            qs = slice(QB * j + n0, QB * j + QB)
                        W = QB - n0
                        ps_s = psSc.tile([128, QB], F32, tag="sc", name=f"sc_{j}_{h}_{t}")
                        nc.tensor.matmul(
                            ps_s[:, 0:W], kt_sb[:, ks], qt_sb[:, h, qs],
                            start=True, stop=True,
                        )
                        if diag_m >= 0:
                            nc.vector.tensor_add(
                                ps_s[:, 0:128], ps_s[:, 0:128], tri_sb[:]
                            )
                        pt = t3.tile([128, QB], BF16, tag="pt", name=f"pt_{j}_{h}_{t}")
                        nc.scalar.activation(
                            pt[:, 0:W], ps_s[:, 0:W], AF.Exp, scale=0.125
                        )
                        nc.tensor.matmul(
                            pv[0 : HD + 1, n0:QB], vaug_sb[:, t, :], pt[:, 0:W],
                            start=(t == 0), stop=(t == ntile - 1),
                        )
                    att = t3.tile([HD + 1, QB], BF16, tag="att", name=f"att_{j}_{h}")
                    nc.vector.tensor_copy(att[:], pv[:])
                    for cc in range(QB // ROWS):
                        s = (QB // ROWS) * (j % 2) + cc
                        cs = slice(ROWS * cc, ROWS * (cc + 1))
                        nc.sync.dma_start(a2a_in[half][s, h, :, :], att[:, cs])
                    # interleave half-0 out-projection into block-3 PE slack
                    if j == 3:
                        p5_matmul(0, h)

                if j == 1:
                    nc.gpsimd.collective_compute(
                        "AllToAll",
                        mybir.AluOpType.bypass,
                        replica_groups=[list(range(N_CORES))],
                        ins=[a2a_in[0][:].opt()],
                        outs=[a2a_out[0][:].opt()],
                    )

            # ---------------- tail: half 1 ----------------
            nc.gpsimd.collective_compute(
                "AllToAll",
                mybir.AluOpType.bypass,
                replica_groups=[list(range(N_CORES))],
                ins=[a2a_in[1][:].opt()],
                outs=[a2a_out[1][:].opt()],
            )
            p5_prep(1)
            for nb in range(4):
                p5_matmul(1, nb)

    nc.compile()
    return nc


_NC_CACHE = None


def _get_nc():
    global _NC_CACHE
    if _NC_CACHE is None:
        _NC_CACHE = _build()
    return _NC_CACHE


def _make_in_maps(x, cos, sin, wq, wk, wv, wo, q_norm_w, k_norm_w):
    x = np.asarray(x, dtype=np.float32)
    cos = np.asarray(cos, dtype=np.float32)
    sin = np.asarray(sin, dtype=np.float32)
    wq = np.asarray(wq, dtype=np.float32)
    wk = np.asarray(wk, dtype=np.float32)
    wv = np.asarray(wv, dtype=np.float32)
    wo = np.asarray(wo, dtype=np.float32)
    qw = np.asarray(q_norm_w, dtype=np.float32)
    kw = np.asarray(k_norm_w, dtype=np.float32)

    # x re-tiled: xq[p, j, k, c] = x[0][512j+c, 128k+p]
    xh = x[0].astype(BFNP)  # [SEQ, D_IN]
    xq_t = np.ascontiguousarray(
        xh.reshape(NQB, QB, KT, 128).transpose(3, 0, 2, 1)
    )  # [128, NQB, KT, QB]

    # weight-folded rope tables (signs of rotate-half folded into sin)
    cosT = cos.T  # [64, SEQ]
    sinT = sin.T
    sgn = np.concatenate([-np.ones(32, np.float32), np.ones(32, np.float32)])

    def fold(w):
        w_rot = np.concatenate([w[32:], w[:32]])
        c64 = cosT * w[:, None]
        s64 = sinT * (sgn * w_rot)[:, None]
        return c64, s64

    qc64, qs64 = fold(qw)
    kc64, ks64 = fold(kw)
    cosq_h = np.ascontiguousarray(np.vstack([qc64, qc64]).astype(BFNP))
    sinq_h = np.ascontiguousarray(np.vstack([qs64, qs64]).astype(BFNP))
    cosk_h = np.ascontiguousarray(kc64.astype(BFNP))
    sink_h = np.ascontiguousarray(ks64.astype(BFNP))

    ii, jj = np.meshgrid(np.arange(128), np.arange(128), indexing="ij")
    tri_h = np.where(ii <= jj, 0.0, NEG).astype(np.float32)  # keep kv<=q
    onesblk_h = np.zeros((128, 128), np.float32)
    onesblk_h[0:64, 0:64] = 1.0
    onesblk_h[64:128, 64:128] = 1.0
    onesblk_h = onesblk_h.astype(BFNP)
    onescol_h = np.ones((128, 1), np.float32).astype(BFNP)
    ident_h = np.eye(64, dtype=np.float32).astype(BFNP)
    sel_h = np.zeros((4 * N_CORES, 2 * N_CORES, 128), np.float32)
    for g in range(N_CORES):
        for u in range(2):
            for m in range(128):
                sel_h[4 * g + 2 * u + m // 64, 2 * g + u, m] = 1.0
    sel_h = sel_h.astype(BFNP)

    woh = np.ascontiguousarray(
        wo.reshape(KT, 128, D_IN).transpose(1, 0, 2).astype(BFNP)
    )

    in_maps = []
    for c in range(N_CORES):
        wq_c = wq[:, 256 * c : 256 * c + 256]
        wq_c = np.ascontiguousarray(
            wq_c.reshape(KT, 128, 256).transpose(1, 0, 2).astype(BFNP)
        )
        wkv_c = np.concatenate(
            [wk[:, 64 * c : 64 * c + 64], wv[:, 64 * c : 64 * c + 64]], axis=1
        )
        wkv_c = np.ascontiguousarray(
            wkv_c.reshape(KT, 128, 128).transpose(1, 0, 2).astype(BFNP)
        )
        in_maps.append(
            {
                "xq": xq_t,
                "wq": wq_c,
                "wkv": wkv_c,
                "wo": woh,
                "cosq": cosq_h,
                "sinq": sinq_h,
                "cosk": cosk_h,
                "sink": sink_h,
                "tri": tri_h,
                "onesblk": onesblk_h,
                "onescol": onescol_h,
                "ident": ident_h,
                "sel": sel_h,
            }
        )
    return in_maps


def kernel(x, cos, sin, wq, wk, wv, wo, q_norm_w, k_norm_w):
    in_maps = _make_in_maps(x, cos, sin, wq, wk, wv, wo, q_norm_w, k_norm_w)
    nc = _get_nc()
    res = run_bass_kernel_spmd(nc, in_maps, core_ids=list(range(N_CORES)))
    full = np.empty((SEQ, D_IN), np.float32)
    for c in range(N_CORES):
        oc = res.results[c]["out"]
        full[128 * c : 128 * c + 128] = oc[0:128]
        full[1024 + 128 * c : 1024 + 128 * c + 128] = oc[128:256]
    return full.reshape(1, SEQ, D_IN).astype(np.float32)


# revision 39
# speedup vs baseline: 1.4551x; 1.1134x over previous
"""GQA FlashAttention (RMSNorm QK + RoPE, causal) on 8 TRN2 NeuronCores.

Sharding: tensor-parallel over heads (core c owns q-heads 4c..4c+3 and
kv-head c; the GQA group is fully local so attention needs no
collective). Attention output is normalized on the producing core
(denominators come free from a ones-column appended to V; the
reciprocal is a fast DVE approximation broadcast across the head dim
by a rank-1 matmul), then re-sharded head-parallel -> row-parallel
with TWO AllToAlls (one per 1024-row half) so the first collective and
the first half of the out-projection overlap with the attention
compute of the second half. Each core then multiplies its 256 output
rows against the full Wo held in SBUF (bf16, prefetched during the
projection phase).

The projection matmuls for block j+1 are emitted interleaved into the
attention tile loop of block j: attention is scalar-engine(exp)-bound,
and the interleave keeps the tensor engine busy enough that the HAM
clock gate stays at full rate. rsqrt is computed as exp(-0.5*ln(x)) so
the scalar engine stays on ONE activation table set for the whole
kernel. All matmuls run in bf16 (fp32 PSUM accumulate); everything is
computed in the transposed layout (head_dim on partitions) so the
scores output IS the P^T operand the PV matmul needs. The RMSNorm
weights and the rotate-half signs are folded into the cos/sin tables
host-side.
"""

import sys

sys.path.insert(0, "/opt/trn_rl_repo")

import ml_dtypes
import numpy as np
import concourse.bass as bass  # noqa: F401
import concourse.tile as tile
from concourse import mybir, bacc
from concourse.bass_utils import run_bass_kernel_spmd

N_CORES = 8
D_IN = 2048
SEQ = 2048
N_HEADS = 32
N_KV = 8
HD = 64
HPC = N_HEADS // N_CORES  # 4 q heads per core
EPS = 1e-6
NEG = -1.0e9

F32 = mybir.dt.float32
BF16 = mybir.dt.bfloat16
BFNP = ml_dtypes.bfloat16

KT = D_IN // 128  # 16 contraction tiles for projections
QB = 512  # q block
NQB = SEQ // QB  # 4
NKT = SEQ // 128  # 16 kv tiles
ROWS = 128  # output rows per core per half
AF = mybir.ActivationFunctionType


class _OneActSetBacc(bacc.Bacc):
    """Bacc whose activation-table pass maps every activation function to
    the natural_log_exp_and_others set (exp/ln/square/copy all live there),
    so the scalar engine loads its table exactly once instead of thrashing
    between the exp and natural-log sets on every rsqrt."""

    def insert_act_table_loads(self):
        import bass_rust
        from concourse import mybir as _mybir
        from concourse.hw_specs import get_activation_tables

        has_activation = any(
            isinstance(i, _mybir.InstActivation)
            for b in self.main_func.blocks
            for i in b.instructions
        )
        if not has_activation:
            return
        tables = [
            (name, fns if name == "natural_log_exp_and_others" else set())
            for name, fns in get_activation_tables(self.m.arch).items()
        ]
        bass_rust.insert_act_table_loads(self, tables)


def _build():
    nc = _OneActSetBacc(num_devices=N_CORES)

    # x re-tiled host-side: xq[p, j, k, c] = x[512j+c, 128k+p]
    xq = nc.dram_tensor("xq", [128, NQB, KT, QB], BF16, kind="ExternalInput")
    wq = nc.dram_tensor("wq", [128, KT, HPC * HD], BF16, kind="ExternalInput")
    wkv = nc.dram_tensor("wkv", [128, KT, 2 * HD], BF16, kind="ExternalInput")
    wo = nc.dram_tensor("wo", [128, KT, D_IN], BF16, kind="ExternalInput")
    cosq = nc.dram_tensor("cosq", [128, SEQ], BF16, kind="ExternalInput")
    sinq = nc.dram_tensor("sinq", [128, SEQ], BF16, kind="ExternalInput")
    cosk = nc.dram_tensor("cosk", [64, SEQ], BF16, kind="ExternalInput")
    sink = nc.dram_tensor("sink", [64, SEQ], BF16, kind="ExternalInput")
    tri = nc.dram_tensor("tri", [128, 128], F32, kind="ExternalInput")
    onesblk_in = nc.dram_tensor("onesblk", [128, 128], BF16, kind="ExternalInput")
    ident_in = nc.dram_tensor("ident", [64, 64], BF16, kind="ExternalInput")

    out = nc.dram_tensor("out", [2 * ROWS, D_IN], F32, kind="ExternalOutput")

    with tile.TileContext(nc) as tc:
        with (
            tc.tile_pool(name="pers", bufs=1) as pers,
            tc.tile_pool(name="dram", bufs=1, space="DRAM") as dram,
            tc.tile_pool(name="xp", bufs=2) as xp,
            tc.tile_pool(name="psproj", bufs=1, space="PSUM") as psProj,
            tc.tile_pool(name="psaux", bufs=1, space="PSUM") as psAux,
            tc.tile_pool(name="pspo", bufs=1, space="PSUM") as psPo,
            tc.tile_pool(name="pssc", bufs=2, space="PSUM") as psSc,
            tc.tile_pool(name="pspv", bufs=1, space="PSUM") as psPv,
            tc.tile_pool(name="t1", bufs=2) as t1,
            tc.tile_pool(name="t3", bufs=3) as t3,
            tc.tile_pool(name="t5", bufs=2) as t5,
        ):
            # ---------------- persistent SBUF ----------------
            # load order = need order: wq + x block 0 gate the first matmul
            wq_sb = pers.tile([128, KT, HPC * HD], BF16)  # 1 MB
            nc.sync.dma_start(wq_sb[:], wq[:])
            xts = [None] * NQB
            xts[0] = xp.tile([128, KT, QB], BF16, tag="xt", name="xt_0")
            nc.sync.dma_start(xts[0][:], xq[:, 0])
            wkv_sb = pers.tile([128, KT, 2 * HD], BF16)  # 0.5 MB
            nc.sync.dma_start(wkv_sb[:], wkv[:])
            cosq_sb = pers.tile([128, SEQ], BF16)
            sinq_sb = pers.tile([128, SEQ], BF16)
            cosk_sb = pers.tile([64, SEQ], BF16)
            sink_sb = pers.tile([64, SEQ], BF16)
            nc.sync.dma_start(cosq_sb[:], cosq[:])
            nc.sync.dma_start(sinq_sb[:], sinq[:])
            nc.sync.dma_start(cosk_sb[:], cosk[:])
            nc.sync.dma_start(sink_sb[:], sink[:])
            tri_sb = pers.tile([128, 128], F32)
            nc.sync.dma_start(tri_sb[:], tri[:])
            onesblk = pers.tile([128, 128], BF16)
            nc.sync.dma_start(onesblk[:], onesblk_in[:])
            ident = pers.tile([64, 64], BF16)
            nc.sync.dma_start(ident[:], ident_in[:])

            eps_sb = pers.tile([128, 1], F32)
            nc.vector.memset(eps_sb[:], EPS)

            qt_sb = pers.tile([64, HPC, SEQ], BF16)  # 1 MB
            kt_sb = pers.tile([64, SEQ], BF16)
            vaug_sb = pers.tile([128, NKT, HD + 1], BF16)
            nc.vector.memset(vaug_sb[:, :, HD : HD + 1], 1.0)

            # wo prefetch (8 MB bf16), in 4 chunks
            wo_sb = pers.tile([128, KT, D_IN], BF16)
            for wch in range(4):
                nc.sync.dma_start(
                    wo_sb[:, 4 * wch : 4 * wch + 4, :], wo[:, 4 * wch : 4 * wch + 4, :]
                )

            # DRAM scratch for the two AllToAlls (rows = head dims, head h
            # at 64h..64h+64; attention output is pre-normalized)
            a2a_in = [
                dram.tile([N_CORES, 4 * HD, ROWS], BF16, name=f"a2a_in{i}")
                for i in range(2)
            ]
            a2a_out = [
                dram.tile([N_CORES, 4 * HD, ROWS], BF16, name=f"a2a_out{i}")
                for i in range(2)
            ]

            # normalized attnT (head-dim-major) per half, filled by DMA
            an_raw = [
                pers.tile([128, 2 * N_CORES, ROWS], BF16, name=f"an_raw{i}")
                for i in range(2)
            ]

            def p5_prep(half):
                """Pull this core's 128 rows (all 2048 head dims) from the
                AllToAll result."""
                for g in range(N_CORES):
                    nc.sync.dma_start(
                        an_raw[half][:, 2 * g : 2 * g + 2, :],
                        a2a_out[half][g].rearrange("(u p) r -> p u r", u=2),
                    )

            def p5_matmul(half, nb):
                """One 512-col block of the out-projection for one half."""
                osl = slice(512 * nb, 512 * nb + 512)
                po = psPo.tile([128, 512], F32, tag="po", name=f"po_{half}_{nb}")
                for gh in range(2 * N_CORES):
                    nc.tensor.matmul(
                        po[:],
                        an_raw[half][:, gh, :],
                        wo_sb[:, gh, osl],
                        start=(gh == 0),
                        stop=(gh == 2 * N_CORES - 1),
                    )
                osb = t5.tile([128, 512], F32, tag="osb", name=f"osb_{half}_{nb}")
                nc.vector.tensor_copy(osb[:], po[:])
                nc.sync.dma_start(out[128 * half : 128 * half + 128, osl], osb[:])

            def emit_proj(j):
                """Projection matmul emitters for block j (3 accs x 16 k)."""
                acc = [
                    psProj.tile([128, QB], F32, tag="acc0", name=f"acc0_{j}"),
                    psProj.tile([128, QB], F32, tag="acc1", name=f"acc1_{j}"),
                    psProj.tile([128, QB], F32, tag="acc2", name=f"acc2_{j}"),
                ]
                xt = xts[j]
                ops = []
                for k in range(KT):
                    st = k == 0
                    sp = k == KT - 1
                    ops.append(
                        lambda k=k, st=st, sp=sp: (
                            nc.tensor.matmul(
                                acc[0][:], wq_sb[:, k, 0:128], xt[:, k, :],
                                start=st, stop=sp,
                            ),
                            nc.tensor.matmul(
                                acc[1][:], wq_sb[:, k, 128:256], xt[:, k, :],
                                start=st, stop=sp,
                            ),
                            nc.tensor.matmul(
                                acc[2][:], wkv_sb[:, k, :], xt[:, k, :],
                                start=st, stop=sp,
                            ),
                        )
                    )
                return acc, ops

            def norm_rope(j, acc):
                """RMSNorm + RoPE for block j (kv first: attention block j
                needs K before the later q heads)."""
                sl = slice(QB * j, QB * j + QB)
                for idx in (2, 0, 1):
                    raw = acc[idx]
                    is_kv = idx == 2
                    nr = 64 if is_kv else 128
                    rows = slice(0, nr)
                    sq = t1.tile([128, QB], BF16, tag="sq", name=f"sq_{j}_{idx}")
                    nc.scalar.activation(sq[rows, :], raw[rows, :], AF.Square)
                    psn = psAux.tile([128, QB], F32, tag="aux", name=f"psn_{j}_{idx}")
                    nc.tensor.matmul(
                        psn[rows, :], onesblk[rows, rows], sq[rows, :],
                        start=True, stop=True,
                    )
                    # rsqrt(ms + eps) = exp(-0.5 * ln(ms + eps)); one ACT table set
                    lnt = t1.tile([128, QB], F32, tag="lnt", name=f"lnt_{j}_{idx}")
                    nc.scalar.activation(
                        lnt[rows, :], psn[rows, :], AF.Ln, bias=eps_sb[rows, :],
                        scale=1.0 / HD,
                    )
                    rcp = t1.tile([128, QB], F32, tag="rcp", name=f"rcp_{j}_{idx}")
                    nc.scalar.activation(rcp[rows, :], lnt[rows, :], AF.Exp, scale=-0.5)
                    tn = t1.tile([128, QB], BF16, tag="tn", name=f"tn_{j}_{idx}")
                    nc.vector.tensor_mul(tn[rows, :], raw[rows, :], rcp[rows, :])
                    # rotate-half (signs folded into the sin tables)
                    rot = t1.tile([128, QB], BF16, tag="rot", name=f"rot_{j}_{idx}")
                    for b in range(1 if is_kv else 2):
                        o = 64 * b
                        nc.vector.tensor_copy(rot[o : o + 32, :], tn[o + 32 : o + 64, :])
                        nc.vector.tensor_copy(rot[o + 32 : o + 64, :], tn[o : o + 32, :])
                    cw = cosk_sb[:, sl] if is_kv else cosq_sb[rows, sl]
                    sw = sink_sb[:, sl] if is_kv else sinq_sb[rows, sl]
                    tmpc = t1.tile([128, QB], BF16, tag="tmpc", name=f"tmpc_{j}_{idx}")
                    nc.vector.tensor_mul(tmpc[rows, :], tn[rows, :], cw)
                    nc.vector.tensor_mul(rot[rows, :], rot[rows, :], sw)
                    if is_kv:
                        nc.vector.tensor_add(kt_sb[:, sl], tmpc[0:64, :], rot[0:64, :])
                        # V: evict + transpose to kv-major layout
                        vt = t1.tile([64, QB], BF16, tag="vt", name=f"vt_{j}")
                        nc.vector.tensor_copy(vt[:], raw[64:128, :])
                        for ttl in range(QB // 128):
                            tg = (QB // 128) * j + ttl
                            psv = psAux.tile(
                                [128, HD], BF16, tag="aux", name=f"psv_{tg}"
                            )
                            nc.tensor.transpose(
                                psv[:], vt[:, 128 * ttl : 128 * ttl + 128], ident[:]
                            )
                            nc.vector.tensor_copy(vaug_sb[:, tg, 0:HD], psv[:])
                    else:
                        for b in range(2):
                            nc.vector.tensor_add(
                                qt_sb[:, 2 * idx + b, sl],
                                tmpc[64 * b : 64 * b + 64, :],
                                rot[64 * b : 64 * b + 64, :],
                            )

            def attention(j, filler):
                """Attention block j; pops PE filler ops (next block's
                projections) between tiles to keep the tensor engine dense."""
                ntile = (QB // 128) * (j + 1)
                half = j // 2
                s0 = (QB // ROWS) * (j % 2)
                n_stops = HPC * ntile
                fi = 0
                stop = 0
                for h in range(HPC):
                    pv = psPv.tile([HD + 1, QB], F32, tag="pv", name=f"pv_{j}_{h}")
                    for t in range(ntile):
                        diag_m = t - (QB // 128) * j
                        ks = slice(128 * t, 128 * t + 128)
                        if diag_m < 0:
                            n0 = 0
                            qs = slice(QB * j, QB * j + QB)
                        else:
                            n0 = 128 * diag_m
                            qs = slice(QB * j + n0, QB * j + QB)
                        W = QB - n0
                        ps_s = psSc.tile([128, QB], F32, tag="sc", name=f"sc_{j}_{h}_{t}")
                        nc.tensor.matmul(
                            ps_s[:, 0:W], kt_sb[:, ks], qt_sb[:, h, qs],
                            start=True, stop=True,
                        )
                        if diag_m >= 0:
                            nc.vector.tensor_add(
                                ps_s[:, 0:128], ps_s[:, 0:128], tri_sb[:]
                            )
                        pt = t3.tile([128, QB], BF16, tag="pt", name=f"pt_{j}_{h}_{t}")
                        nc.scalar.activation(
                            pt[:, 0:W], ps_s[:, 0:W], AF.Exp, scale=0.125
                        )
                        nc.tensor.matmul(
                            pv[0 : HD + 1, n0:QB], vaug_sb[:, t, :], pt[:, 0:W],
                            start=(t == 0), stop=(t == ntile - 1),
                        )
                        stop += 1
                        while fi < len(filler) and fi * n_stops < stop * len(filler):
                            filler[fi]()
                            fi += 1
                    # normalize on the producing core: denominator reciprocal
                    # (fast approx) broadcast across the head dim by a rank-1
                    # matmul, then one multiply
                    dcp = t3.tile([1, QB], F32, tag="dcp", name=f"dcp_{j}_{h}")
                    nc.vector.tensor_copy(dcp[:], pv[HD : HD + 1, :])
                    rden = t3.tile([1, QB], F32, tag="rden", name=f"rden_{j}_{h}")
                    # (approx-recip's const operands live at partition 0 - the
                    # input must too)
                    nc.vector.reciprocal_approx_fast(rden[:], dcp[:])
                    rdb = t3.tile([1, QB], BF16, tag="rdb", name=f"rdb_{j}_{h}")
                    nc.vector.tensor_copy(rdb[:], rden[:])
                    bc = psAux.tile([HD, QB], F32, tag="aux", name=f"bc_{j}_{h}")
                    nc.tensor.matmul(
                        bc[:], onesblk[0:1, 0:HD], rdb[:], start=True, stop=True
                    )
                    att = t3.tile([HD, QB], BF16, tag="att", name=f"att_{j}_{h}")
                    nc.vector.tensor_copy(att[:], pv[0:HD, :])
                    atn = t3.tile([HD, QB], BF16, tag="atn", name=f"atn_{j}_{h}")
                    nc.vector.tensor_mul(atn[:], att[:], bc[:])
                    # one DMA per (j, h): 4 shard chunks at once
                    nc.sync.dma_start(
                        a2a_in[half][s0 : s0 + 4, 64 * h : 64 * h + 64, :].transpose(
                            [1, 0, 2]
                        ),
                        atn[:].rearrange("p (cc r) -> p cc r", cc=4),
                    )
                while fi < len(filler):
                    filler[fi]()
                    fi += 1

            # ---------------- prologue: block 0 ----------------
            xts[1] = xp.tile([128, KT, QB], BF16, tag="xt", name="xt_1")
            nc.sync.dma_start(xts[1][:], xq[:, 1])
            acc_j, ops = emit_proj(0)
            for op in ops:
                op()
            norm_rope(0, acc_j)

            # ---------------- pipeline ----------------
            for j in range(NQB):
                if j + 1 < NQB:
                    next_acc, filler = emit_proj(j + 1)
                else:
                    next_acc, filler = None, []
                attention(j, filler)
                if j + 2 < NQB:
                    xts[j + 2] = xp.tile(
                        [128, KT, QB], BF16, tag="xt", name=f"xt_{j + 2}"
                    )
                    nc.sync.dma_start(xts[j + 2][:], xq[:, j + 2])
                if next_acc is not None:
                    norm_rope(j + 1, next_acc)
                if j == 1:
                    nc.gpsimd.collective_compute(
                        "AllToAll",
                        mybir.AluOpType.bypass,
                        replica_groups=[list(range(N_CORES))],
                        ins=[a2a_in[0][:].opt()],
                        outs=[a2a_out[0][:].opt()],
                    )
                if j == 2:
                    p5_prep(0)

            # ---------------- tail ----------------
            nc.gpsimd.collective_compute(
                "AllToAll",
                mybir.AluOpType.bypass,
                replica_groups=[list(range(N_CORES))],
                ins=[a2a_in[1][:].opt()],
                outs=[a2a_out[1][:].opt()],
            )
            # half-0 out-projection fills the PE during the second AllToAll;
            # the wait hint stops the scheduler from hoisting it into block 3
            with tc.tile_wait_until(0.2):
                for nb in range(4):
                    p5_matmul(0, nb)
            p5_prep(1)
            for nb in range(4):
                p5_matmul(1, nb)

    nc.compile()
    return nc


_NC_CACHE = None


def _get_nc():
    global _NC_CACHE
    if _NC_CACHE is None:
        _NC_CACHE = _build()
    return _NC_CACHE


def _make_in_maps(x, cos, sin, wq, wk, wv, wo, q_norm_w, k_norm_w):
    x = np.asarray(x, dtype=np.float32)
    cos = np.asarray(cos, dtype=np.float32)
    sin = np.asarray(sin, dtype=np.float32)
    wq = np.asarray(wq, dtype=np.float32)
    wk = np.asarray(wk, dtype=np.float32)
    wv = np.asarray(wv, dtype=np.float32)
    wo = np.asarray(wo, dtype=np.float32)
    qw = np.asarray(q_norm_w, dtype=np.float32)
    kw = np.asarray(k_norm_w, dtype=np.float32)

    # x re-tiled: xq[p, j, k, c] = x[0][512j+c, 128k+p]
    xh = x[0].astype(BFNP)  # [SEQ, D_IN]
    xq_t = np.ascontiguousarray(
        xh.reshape(NQB, QB, KT, 128).transpose(3, 0, 2, 1)
    )  # [128, NQB, KT, QB]

    # weight-folded rope tables (signs of rotate-half folded into sin)
    cosT = cos.T  # [64, SEQ]
    sinT = sin.T
    sgn = np.concatenate([-np.ones(32, np.float32), np.ones(32, np.float32)])

    def fold(w):
        w_rot = np.concatenate([w[32:], w[:32]])
        c64 = cosT * w[:, None]
        s64 = sinT * (sgn * w_rot)[:, None]
        return c64, s64

    qc64, qs64 = fold(qw)
    kc64, ks64 = fold(kw)
    cosq_h = np.ascontiguousarray(np.vstack([qc64, qc64]).astype(BFNP))
    sinq_h = np.ascontiguousarray(np.vstack([qs64, qs64]).astype(BFNP))
    cosk_h = np.ascontiguousarray(kc64.astype(BFNP))
    sink_h = np.ascontiguousarray(ks64.astype(BFNP))

    ii, jj = np.meshgrid(np.arange(128), np.arange(128), indexing="ij")
    tri_h = np.where(ii <= jj, 0.0, NEG).astype(np.float32)  # keep kv<=q
    onesblk_h = np.zeros((128, 128), np.float32)
    onesblk_h[0:64, 0:64] = 1.0
    onesblk_h[64:128, 64:128] = 1.0
    onesblk_h = onesblk_h.astype(BFNP)
    ident_h = np.eye(64, dtype=np.float32).astype(BFNP)

    woh = np.ascontiguousarray(
        wo.reshape(KT, 128, D_IN).transpose(1, 0, 2).astype(BFNP)
    )

    in_maps = []
    for c in range(N_CORES):
        wq_c = wq[:, 256 * c : 256 * c + 256]
        wq_c = np.ascontiguousarray(
            wq_c.reshape(KT, 128, 256).transpose(1, 0, 2).astype(BFNP)
        )
        wkv_c = np.concatenate(
            [wk[:, 64 * c : 64 * c + 64], wv[:, 64 * c : 64 * c + 64]], axis=1
        )
        wkv_c = np.ascontiguousarray(
            wkv_c.reshape(KT, 128, 128).transpose(1, 0, 2).astype(BFNP)
        )
        in_maps.append(
            {
                "xq": xq_t,
                "wq": wq_c,
                "wkv": wkv_c,
                "wo": woh,
                "cosq": cosq_h,
                "sinq": sinq_h,
                "cosk": cosk_h,
                "sink": sink_h,
                "tri": tri_h,
                "onesblk": onesblk_h,
                "ident": ident_h,
            }
        )
    return in_maps


def kernel(x, cos, sin, wq, wk, wv, wo, q_norm_w, k_norm_w):
    in_maps = _make_in_maps(x, cos, sin, wq, wk, wv, wo, q_norm_w, k_norm_w)
    nc = _get_nc()
    res = run_bass_kernel_spmd(nc, in_maps, core_ids=list(range(N_CORES)))
    full = np.empty((SEQ, D_IN), np.float32)
    for c in range(N_CORES):
        oc = res.results[c]["out"]
        full[128 * c : 128 * c + 128] = oc[0:128]
        full[1024 + 128 * c : 1024 + 128 * c + 128] = oc[128:256]
    return full.reshape(1, SEQ, D_IN).astype(np.float32)
